# revision 11
# baseline (speedup 1.0000x reference)
"""Multi-Head Latent Attention (MLA) Bass kernel for 8 trn2 NeuronCores.

Sharding: core c handles batch b=c//4 and head group hg=c%4 (4 of 16 heads).
Host transposes x[b] once and pre-casts everything to bf16; the device
pipeline runs in "transposed" layout (feature dims on SBUF partitions).

v4 design (HAM-warmth focused: PE must never idle, so the 2.4GHz clock
gate stays open; v3 ran effectively at the cold 1.2GHz rate):
  - q-path FOLDED on host (A_h = W_dq @ W_uq_h); kv-path two-stage.
  - flat attention schedule over (head, chunk) with a 2-chunk qk lookahead
    that crosses head boundaries, so av(c) never stalls PE on exp(c).
  - one shared 4-bank PSUM rotation ("ps") for scores / projection chains /
    recip-broadcasts / W_o tiles + 2 av banks + 2 sum banks = 8 banks.
  - per-head normalization (recip -> broadcast matmul -> avn mul) deferred
    2 chunks into the next head so its DVE chain is off the PE path.
  - block order: att(t) -> ckv(t+1) -> norm(h3,t) -> W_o(t) -> proj rest
    (t+1); W_o contracts h=3 last so it can start before avn(3) lands.
  - rope: ACT copies PSUM->SBUF once; rotate-half shuffle on idle GPSIMD;
    muls/add on DVE with bf16 cos/sin tables.
  - bf16 out (host upcasts+sums); weights spread over 4 DMA queues.
"""

import numpy as np

T = 2048
C = 2048
QC = 1536
KV = 512
NH = 16
DH = 128
R = 64
TB = 512           # time block / q-group width
NTB = T // TB      # 4
SCALE = 1.0 / float(np.sqrt(DH + R))
ROPE_BASE = 10000.0

_CACHE = {}


def _build_nc(repeat=1):
    import concourse.bacc as bacc
    import concourse.mybir as mybir
    import concourse.tile as tile

    BF16 = mybir.dt.bfloat16

    nc = bacc.Bacc("TRN2", target_bir_lowering=False, debug=False)

    xT = nc.dram_tensor("xT", [C, T], BF16, kind="ExternalInput")
    aq = nc.dram_tensor("aq", [C, 512], BF16, kind="ExternalInput")
    aqr = nc.dram_tensor("aqr", [C, 256], BF16, kind="ExternalInput")
    wdkv = nc.dram_tensor("wdkv", [C, KV], BF16, kind="ExternalInput")
    wuk = nc.dram_tensor("wuk", [KV, 512], BF16, kind="ExternalInput")
    wuv = nc.dram_tensor("wuv", [KV, 512], BF16, kind="ExternalInput")
    wkr = nc.dram_tensor("wkr", [KV, 256], BF16, kind="ExternalInput")
    wo = nc.dram_tensor("wo", [512, C], BF16, kind="ExternalInput")
    cosd = nc.dram_tensor("cosd", [128, T], BF16, kind="ExternalInput")
    sind = nc.dram_tensor("sind", [128, T], BF16, kind="ExternalInput")
    maskd = nc.dram_tensor("maskd", [128, 128], BF16, kind="ExternalInput")
    onesd = nc.dram_tensor("onesd", [128, 128], BF16, kind="ExternalInput")
    out = nc.dram_tensor("out", [T, C], BF16, kind="ExternalOutput")

    with tile.TileContext(nc) as tc:
        for _rep in range(repeat):
            _emit_body(nc, tc, mybir,
                       xT, aq, aqr, wdkv, wuk, wuv, wkr, wo,
                       cosd, sind, maskd, onesd, out)

    nc.compile()
    return nc


def _emit_body(nc, tc, mybir,
               xT, aq, aqr, wdkv, wuk, wuv, wkr, wo,
               cosd, sind, maskd, onesd, out):
    BF16 = mybir.dt.bfloat16
    F32 = mybir.dt.float32
    AF = mybir.ActivationFunctionType

    def ecopy(eng, dst, src_):
        (eng.copy if eng is nc.scalar else eng.tensor_copy)(dst, src_)

    with (
        tc.tile_pool(name="p1", bufs=1) as sp,
        tc.tile_pool(name="p1ps", bufs=1, space="PSUM") as pp,
    ):
        # ---- constant / weight loads, spread over 4 DGE queues ----
        cos_sb = sp.tile([128, T], BF16, name="cos_sb")
        nc.sync.dma_start(cos_sb[:], cosd[:])
        sin_sb = sp.tile([128, T], BF16, name="sin_sb")
        nc.sync.dma_start(sin_sb[:], sind[:])
        mask_sb = sp.tile([128, 128], BF16, name="mask_sb")
        nc.sync.dma_start(mask_sb[:], maskd[:])
        ones_sb = sp.tile([128, 128], BF16, name="ones_sb")
        nc.sync.dma_start(ones_sb[:], onesd[:])
        wdkv_sb = sp.tile([128, 16, KV], BF16, name="wdkv_sb")
        nc.scalar.dma_start(wdkv_sb[:], wdkv.rearrange("(k p) n -> p k n", p=128))
        aq_sb = sp.tile([128, 16, 512], BF16, name="aq_sb")
        nc.scalar.dma_start(aq_sb[:], aq.rearrange("(k p) n -> p k n", p=128))
        aqr_sb = sp.tile([128, 16, 256], BF16, name="aqr_sb")
        nc.scalar.dma_start(aqr_sb[:], aqr.rearrange("(k p) n -> p k n", p=128))
        wuk_sb = sp.tile([128, 4, 512], BF16, name="wuk_sb")
        nc.gpsimd.dma_start(wuk_sb[:], wuk.rearrange("(k p) n -> p k n", p=128))
        wuv_sb = sp.tile([128, 4, 512], BF16, name="wuv_sb")
        nc.gpsimd.dma_start(wuv_sb[:], wuv.rearrange("(k p) n -> p k n", p=128))
        wkr_sb = sp.tile([128, 4, 256], BF16, name="wkr_sb")
        nc.gpsimd.dma_start(wkr_sb[:], wkr.rearrange("(k p) n -> p k n", p=128))
        wo_sb = sp.tile([128, 4, C], BF16, name="wo_sb")
        nc.gpsimd.dma_start(wo_sb[:], wo.rearrange("(h p) n -> p h n", p=128))

        kc_t = [sp.tile([128, 4, TB], BF16, name=f"kc{t}") for t in range(NTB)]
        kr_t = [sp.tile([128, 2, TB], BF16, name=f"kr{t}") for t in range(NTB)]
        v_t = [sp.tile([128, 4, TB], BF16, name=f"v{t}") for t in range(NTB)]

        def load_xblk(t):
            xb = sp.tile([128, 16, TB], BF16, name="xblk", tag="xblk", bufs=2)
            tc0 = TB * t
            nc.sync.dma_start(
                xb[:], xT[:, tc0:tc0 + TB].rearrange("(k p) n -> p k n", p=128))
            return xb

        def rope_store(ps_t, dst, cs, sn):
            # ps_t [128, TB] PSUM: rows [64 head 2p | 64 head 2p+1] rope dims
            r16 = sp.tile([128, TB], BF16, name="r16", tag="r16", bufs=2)
            nc.scalar.copy(r16[:], ps_t[:])
            t1 = sp.tile([128, TB], BF16, name="rp1", tag="rp1", bufs=2)
            nc.vector.tensor_mul(t1[:], r16[:], cs)
            sh = sp.tile([128, TB], BF16, name="rp2", tag="rp2", bufs=2)
            nc.vector.tensor_copy(sh[0:32, :], r16[32:64, :])
            nc.vector.tensor_copy(sh[32:64, :], r16[0:32, :])
            nc.vector.tensor_copy(sh[64:96, :], r16[96:128, :])
            nc.vector.tensor_copy(sh[96:128, :], r16[64:96, :])
            nc.vector.tensor_mul(sh[:], sh[:], sn)
            nc.vector.tensor_add(dst, t1[:], sh[:])

        def proj_ckv(t, xb):
            ckvb = sp.tile([128, 4, TB], BF16, name="ckv_blk", tag="ckv", bufs=2)
            for m in range(4):
                ps_t = pp.tile([128, TB], F32, name="ps_p", tag="ps", bufs=4)
                for k in range(16):
                    nc.tensor.matmul(ps_t[:], wdkv_sb[:, k, 128 * m:128 * (m + 1)],
                                     xb[:, k, :], start=(k == 0), stop=(k == 15))
                eng = nc.scalar if m % 2 == 0 else nc.vector
                ecopy(eng, ckvb[:, m, :], ps_t[:])
            return ckvb

        def proj_rest(t, xb, ckvb):
            tc0 = TB * t
            cs = cos_sb[:, tc0:tc0 + TB]
            sn = sin_sb[:, tc0:tc0 + TB]
            qcb = sp.tile([128, 4, TB], BF16, name="qc_blk", tag="qc", bufs=2)
            for h in range(4):
                ps_t = pp.tile([128, TB], F32, name="ps_p", tag="ps", bufs=4)
                for k in range(16):
                    nc.tensor.matmul(
                        ps_t[:], aq_sb[:, k, 128 * h:128 * (h + 1)],
                        xb[:, k, :], start=(k == 0), stop=(k == 15))
                eng = nc.scalar if h % 2 == 0 else nc.vector
                ecopy(eng, qcb[:, h, :], ps_t[:])
            qrb = sp.tile([128, 2, TB], BF16, name="qr_blk", tag="qr", bufs=2)
            for p in range(2):
                ps_t = pp.tile([128, TB], F32, name="ps_p", tag="ps", bufs=4)
                for k in range(16):
                    nc.tensor.matmul(
                        ps_t[:], aqr_sb[:, k, 128 * p:128 * (p + 1)],
                        xb[:, k, :], start=(k == 0), stop=(k == 15))
                rope_store(ps_t, qrb[:, p, :], cs, sn)
            for h in range(4):
                ps_t = pp.tile([128, TB], F32, name="ps_p", tag="ps", bufs=4)
                for k in range(4):
                    nc.tensor.matmul(
                        ps_t[:], wuk_sb[:, k, 128 * h:128 * (h + 1)],
                        ckvb[:, k, :], start=(k == 0), stop=(k == 3))
                eng = nc.scalar if h % 2 == 0 else nc.vector
                ecopy(eng, kc_t[t][:, h, :], ps_t[:])
            for p in range(2):
                ps_t = pp.tile([128, TB], F32, name="ps_p", tag="ps", bufs=4)
                for k in range(4):
                    nc.tensor.matmul(
                        ps_t[:], wkr_sb[:, k, 128 * p:128 * (p + 1)],
                        ckvb[:, k, :], start=(k == 0), stop=(k == 3))
                rope_store(ps_t, kr_t[t][:, p, :], cs, sn)
            for tkc in range(4):
                ps_t = pp.tile([128, TB], F32, name="ps_p", tag="ps", bufs=4)
                for k in range(4):
                    nc.tensor.matmul(
                        ps_t[:], ckvb[:, k, 128 * tkc:128 * (tkc + 1)],
                        wuv_sb[:, k, :], start=(k == 0), stop=(k == 3))
                eng = nc.scalar if tkc % 2 == 0 else nc.vector
                ecopy(eng, v_t[t][:, tkc, :], ps_t[:])
            return qcb, qrb

        def emit_norm(info, avn):
            # recip-broadcast matmul + avn write for a finished head;
            # called once dense PE work sits between it and rec16's producer
            h, ps_av, rec16 = info
            ps_bc = pp.tile([128, TB], F32, name="ps_bc", tag="ps", bufs=4)
            nc.tensor.matmul(ps_bc[:], ones_sb[0:1, :], rec16[:],
                             start=True, stop=True)
            av16 = sp.tile([128, TB], BF16, name="av16", tag="av16", bufs=2)
            nc.scalar.copy(av16[:], ps_av[:])
            nc.vector.tensor_mul(avn[:, TB * h:TB * (h + 1)], av16[:], ps_bc[:])

        def attend(t, qcb, qrb):
            # head pairs (0,1) and (2,3) in lockstep; the two rope matmuls of
            # a pair are adjacent with disjoint row groups (0-63 / 64-127) so
            # the PE runs them concurrently, amortizing the per-MM fixed cost
            nch = 4 * (t + 1)
            avn = sp.tile([128, 4 * TB], BF16, name="avn", tag="avn", bufs=2)
            pending = []
            for pair in range(2):
                heads = (2 * pair, 2 * pair + 1)
                pr = pair

                def qk_pair(c):
                    j = c - 4 * t
                    s = 128 * j if j > 0 else 0
                    blk, jj = divmod(c, 4)
                    res = []
                    for h in heads:
                        ps_s = pp.tile([128, TB], F32, name="ps_s", tag="ps",
                                       bufs=4)
                        nc.tensor.matmul(
                            ps_s[:, s:], kc_t[blk][:, h, 128 * jj:128 * (jj + 1)],
                            qcb[:, h, s:], start=True, stop=False)
                        res.append(ps_s)
                    for ps_s, h in zip(res, heads):
                        p0 = 64 * (h % 2)
                        nc.tensor.matmul(
                            ps_s[:, s:],
                            kr_t[blk][p0:p0 + 64, pr, 128 * jj:128 * (jj + 1)],
                            qrb[p0:p0 + 64, pr, s:], start=False, stop=True)
                    return res

                q = [qk_pair(0)]
                if nch > 1:
                    q.append(qk_pair(1))
                ps_av, ps_sum, dacc = {}, {}, {}
                for c in range(nch):
                    curpair = q.pop(0)
                    j = c - 4 * t
                    s = 128 * j if j > 0 else 0
                    blk, jj = divmod(c, 4)
                    exs_pair = []
                    for idx, h in enumerate(heads):
                        ex = sp.tile([128, TB], BF16, name="ex", tag="ex",
                                     bufs=8)
                        nc.scalar.activation(ex[:, s:], curpair[idx][:, s:],
                                             AF.Exp, scale=SCALE)
                        if j >= 0:
                            nc.vector.tensor_mul(ex[:, s:s + 128],
                                                 ex[:, s:s + 128], mask_sb[:])
                        exs_pair.append(ex)
                    if c + 2 < nch:
                        q.append(qk_pair(c + 2))
                    if c == 0:
                        for h in heads:
                            ps_av[h] = pp.tile([128, TB], F32, name="ps_av",
                                               tag="ps_av", bufs=2)
                            ps_sum[h] = pp.tile([1, TB], F32, name="ps_sum",
                                                tag="ps_sum", bufs=2)
                        for info in pending:
                            emit_norm(info, avn)
                        pending = []
                    for idx, h in enumerate(heads):
                        nc.tensor.matmul(
                            ps_av[h][:, s:],
                            v_t[blk][:, jj, 128 * h:128 * (h + 1)],
                            exs_pair[idx][:, s:],
                            start=(c == 0), stop=(c == nch - 1))
                    for idx, h in enumerate(heads):
                        # DVE-accumulate every chunk's exp so the denominator
                        # costs ONE ones-matmul per head
                        ex = exs_pair[idx]
                        if c == 0:
                            dacc[h] = sp.tile([128, TB], BF16, name="dacc",
                                              tag="dacc", bufs=2)
                            nc.vector.tensor_copy(dacc[h][:], ex[:])
                        else:
                            nc.vector.tensor_add(dacc[h][:, s:],
                                                 dacc[h][:, s:], ex[:, s:])
                        if c == nch - 1:
                            nc.tensor.matmul(ps_sum[h][:], ones_sb[:, 0:1],
                                             dacc[h][:], start=True, stop=True)
                            recip = sp.tile([1, TB], F32, name="recip",
                                            tag="recip", bufs=2)
                            nc.vector.reciprocal(recip[:], ps_sum[h][:])
                            rec16 = sp.tile([1, TB], BF16, name="rec16",
                                            tag="rec16", bufs=2)
                            nc.vector.tensor_copy(rec16[:], recip[:])
                            pending.append((h, ps_av[h], rec16))
            return avn, pending

        def emit_wo(t, avn):
            tc0 = TB * t
            for tqc in range(4):
                for n in range(4):
                    ps_o = pp.tile([128, 512], F32, name="ps_o", tag="ps",
                                   bufs=4)
                    for h in range(4):
                        nc.tensor.matmul(
                            ps_o[:],
                            avn[:, TB * h + 128 * tqc:TB * h + 128 * (tqc + 1)],
                            wo_sb[:, h, 512 * n:512 * (n + 1)],
                            start=(h == 0), stop=(h == 3))
                    ost = sp.tile([128, 512], BF16, name="ost", tag="ost",
                                  bufs=3)
                    eng = nc.scalar if n % 2 == 0 else nc.vector
                    ecopy(eng, ost[:], ps_o[:])
                    nc.sync.dma_start(
                        out[tc0 + 128 * tqc:tc0 + 128 * (tqc + 1),
                            512 * n:512 * (n + 1)], ost[:])

        # ---- body schedule ----
        xb = load_xblk(0)
        ckvb = proj_ckv(0, xb)
        qcb, qrb = proj_rest(0, xb, ckvb)
        for t in range(NTB):
            if t + 1 < NTB:
                xb_next = load_xblk(t + 1)
            avn, pending = attend(t, qcb, qrb)
            if t + 1 < NTB:
                ckvb = proj_ckv(t + 1, xb_next)
            for info in pending:
                emit_norm(info, avn)
            emit_wo(t, avn)
            if t + 1 < NTB:
                qcb, qrb = proj_rest(t + 1, xb_next, ckvb)


def _rope_tables():
    import ml_dtypes
    inv = 1.0 / (ROPE_BASE ** (np.arange(0, R, 2, dtype=np.float32) / R))
    freqs = np.arange(T, dtype=np.float32)[:, None] * inv[None, :]       # [T, 32]
    emb = np.concatenate([freqs, freqs], axis=-1)                         # [T, 64]
    cosT = np.ascontiguousarray(np.cos(emb).T.astype(np.float32))         # [64, T]
    sinT = np.ascontiguousarray(np.sin(emb).T.astype(np.float32))
    cosd = np.concatenate([cosT, cosT], axis=0)                           # [128, T]
    sin_sgn = np.concatenate([-sinT[0:32], sinT[32:64]], axis=0)          # [64, T]
    sind = np.concatenate([sin_sgn, sin_sgn], axis=0)
    return (cosd.astype(ml_dtypes.bfloat16), sind.astype(ml_dtypes.bfloat16))


def host_inmaps(inputs):
    import ml_dtypes
    BF = ml_dtypes.bfloat16

    x = np.asarray(inputs["x"], dtype=np.float32)
    W_dq = np.asarray(inputs["W_dq"], dtype=np.float32)
    W_uq = np.asarray(inputs["W_uq"], dtype=np.float32)
    W_qr = np.asarray(inputs["W_qr"], dtype=np.float32)
    W_dkv = np.asarray(inputs["W_dkv"], dtype=np.float32).astype(BF)
    W_uk = np.asarray(inputs["W_uk"], dtype=np.float32).astype(BF)
    W_uv = np.asarray(inputs["W_uv"], dtype=np.float32).astype(BF)
    W_kr = np.asarray(inputs["W_kr"], dtype=np.float32).astype(BF)
    W_o = np.asarray(inputs["W_o"], dtype=np.float32).astype(BF)

    Aq = (W_dq @ W_uq).astype(BF)     # [C, NH*DH] folded q-content projection
    Aqr = (W_dq @ W_qr).astype(BF)    # [C, NH*R] folded q-rope projection

    cosd, sind = _rope_tables()
    maskv = (np.arange(128)[:, None] <= np.arange(128)[None, :]).astype(
        np.float32).astype(BF)
    onesv = np.ones((128, 128), dtype=np.float32).astype(BF)

    in_maps = []
    for core in range(8):
        b, hg = core // 4, core % 4
        in_maps.append({
            "xT": np.ascontiguousarray(x[b].T).astype(BF),
            "aq": np.ascontiguousarray(Aq[:, 512 * hg:512 * (hg + 1)]),
            "aqr": np.ascontiguousarray(Aqr[:, 256 * hg:256 * (hg + 1)]),
            "wdkv": W_dkv,
            "wuk": np.ascontiguousarray(W_uk[:, 512 * hg:512 * (hg + 1)]),
            "wuv": np.ascontiguousarray(W_uv[:, 512 * hg:512 * (hg + 1)]),
            "wkr": np.ascontiguousarray(W_kr[:, 256 * hg:256 * (hg + 1)]),
            "wo": np.ascontiguousarray(W_o[512 * hg:512 * (hg + 1), :]),
            "cosd": cosd,
            "sind": sind,
            "maskd": maskv,
            "onesd": onesv,
        })
    return in_maps


def kernel(**inputs):
    from concourse.bass_utils import run_bass_kernel_spmd

    if "nc" not in _CACHE:
        _CACHE["nc"] = _build_nc()
    nc = _CACHE["nc"]

    in_maps = host_inmaps(inputs)

    res = run_bass_kernel_spmd(nc, in_maps, core_ids=list(range(8)))
    outs = [np.asarray(r["out"]).astype(np.float32) for r in res.results]
    out0 = outs[0] + outs[1] + outs[2] + outs[3]
    out1 = outs[4] + outs[5] + outs[6] + outs[7]
    return np.stack([out0, out1]).astype(np.float32)


# revision 13
# speedup vs baseline: 1.0775x; 1.0775x over previous
"""Multi-Head Latent Attention (MLA) Bass kernel for 8 trn2 NeuronCores.

Sharding: core c handles batch b=c//4 and head group hg=c%4 (4 of 16 heads).
Host transposes x[b] once and pre-casts everything to bf16; the device
pipeline runs in "transposed" layout (feature dims on SBUF partitions).

v5 design. Measured reality on this platform (see mmbench3): every
matmul costs ~344ns fixed + 0.39ns/output-col with NO pipelining between
consecutive MMs, so MM instruction count dominates; but matmuls whose
row groups are disjoint (K<=64 at partition offsets 0/64) DO execute
concurrently in the PE sub-arrays. Hence:
  - q-path FOLDED on host (A_h = W_dq @ W_uq_h); kv-path two-stage.
  - attention runs head PAIRS in lockstep; the pair's two rope-score
    matmuls (K=64, rows 0-63 / 64-127) are emitted adjacently and run
    concurrently, halving their fixed cost (~36us/body).
  - 1-chunk-pair qk lookahead keeps exp() latency off the PE path.
  - denominator: quad DVE pre-adds (one ones-matmul per 4 full chunks)
    + DVE-merged diagonal chunks (one ones-matmul for all 4, -48 MMs).
  - one shared 4-bank PSUM rotation ("ps") for scores / projections /
    recip-broadcasts / W_o + 2 av banks + 2 sum banks = 8 banks.
  - per-head normalization (recip -> broadcast matmul -> avn mul)
    deferred into the next pair's prologue, off the PE critical path.
  - block order: att(t) -> ckv(t+1) -> norms -> W_o(t) -> proj rest(t+1);
    W_o contracts h=3 last so it starts before avn(3) lands.
  - rope: ACT copy PSUM->SBUF, DVE rotate-half shuffles + muls, bf16
    cos/sin tables.
  - bf16 out (host upcasts+sums); x/out on SP DMA queue, weights split
    over scalar/gpsimd queues (Pool strict-FIFO is a trap for bulk DMA).
"""

import numpy as np

T = 2048
C = 2048
QC = 1536
KV = 512
NH = 16
DH = 128
R = 64
TB = 512           # time block / q-group width
NTB = T // TB      # 4
SCALE = 1.0 / float(np.sqrt(DH + R))
ROPE_BASE = 10000.0

_CACHE = {}


def _build_nc(repeat=1):
    import concourse.bacc as bacc
    import concourse.mybir as mybir
    import concourse.tile as tile

    BF16 = mybir.dt.bfloat16

    nc = bacc.Bacc("TRN2", target_bir_lowering=False, debug=False)

    xT = nc.dram_tensor("xT", [C, T], BF16, kind="ExternalInput")
    aq = nc.dram_tensor("aq", [C, 512], BF16, kind="ExternalInput")
    aqr = nc.dram_tensor("aqr", [C, 256], BF16, kind="ExternalInput")
    wdkv = nc.dram_tensor("wdkv", [C, KV], BF16, kind="ExternalInput")
    wuk = nc.dram_tensor("wuk", [KV, 512], BF16, kind="ExternalInput")
    wuv = nc.dram_tensor("wuv", [KV, 512], BF16, kind="ExternalInput")
    wkr = nc.dram_tensor("wkr", [KV, 256], BF16, kind="ExternalInput")
    wo = nc.dram_tensor("wo", [512, C], BF16, kind="ExternalInput")
    cosd = nc.dram_tensor("cosd", [128, T], BF16, kind="ExternalInput")
    sind = nc.dram_tensor("sind", [128, T], BF16, kind="ExternalInput")
    maskd = nc.dram_tensor("maskd", [128, 128], BF16, kind="ExternalInput")
    onesd = nc.dram_tensor("onesd", [128, 128], BF16, kind="ExternalInput")
    out = nc.dram_tensor("out", [T, C], BF16, kind="ExternalOutput")

    with tile.TileContext(nc) as tc:
        for _rep in range(repeat):
            _emit_body(nc, tc, mybir,
                       xT, aq, aqr, wdkv, wuk, wuv, wkr, wo,
                       cosd, sind, maskd, onesd, out)

    nc.compile()
    return nc


def _emit_body(nc, tc, mybir,
               xT, aq, aqr, wdkv, wuk, wuv, wkr, wo,
               cosd, sind, maskd, onesd, out):
    BF16 = mybir.dt.bfloat16
    F32 = mybir.dt.float32
    AF = mybir.ActivationFunctionType

    def ecopy(eng, dst, src_):
        (eng.copy if eng is nc.scalar else eng.tensor_copy)(dst, src_)

    with (
        tc.tile_pool(name="p1", bufs=1) as sp,
        tc.tile_pool(name="p1ps", bufs=1, space="PSUM") as pp,
    ):
        # ---- constant / weight loads, spread over 4 DGE queues ----
        cos_sb = sp.tile([128, T], BF16, name="cos_sb")
        nc.sync.dma_start(cos_sb[:], cosd[:])
        sin_sb = sp.tile([128, T], BF16, name="sin_sb")
        nc.sync.dma_start(sin_sb[:], sind[:])
        mask_sb = sp.tile([128, 128], BF16, name="mask_sb")
        nc.sync.dma_start(mask_sb[:], maskd[:])
        ones_sb = sp.tile([128, 128], BF16, name="ones_sb")
        nc.sync.dma_start(ones_sb[:], onesd[:])
        wdkv_sb = sp.tile([128, 16, KV], BF16, name="wdkv_sb")
        nc.scalar.dma_start(wdkv_sb[:], wdkv.rearrange("(k p) n -> p k n", p=128))
        aq_sb = sp.tile([128, 16, 512], BF16, name="aq_sb")
        nc.scalar.dma_start(aq_sb[:], aq.rearrange("(k p) n -> p k n", p=128))
        aqr_sb = sp.tile([128, 16, 256], BF16, name="aqr_sb")
        nc.scalar.dma_start(aqr_sb[:], aqr.rearrange("(k p) n -> p k n", p=128))
        wuk_sb = sp.tile([128, 4, 512], BF16, name="wuk_sb")
        nc.gpsimd.dma_start(wuk_sb[:], wuk.rearrange("(k p) n -> p k n", p=128))
        wuv_sb = sp.tile([128, 4, 512], BF16, name="wuv_sb")
        nc.gpsimd.dma_start(wuv_sb[:], wuv.rearrange("(k p) n -> p k n", p=128))
        wkr_sb = sp.tile([128, 4, 256], BF16, name="wkr_sb")
        nc.gpsimd.dma_start(wkr_sb[:], wkr.rearrange("(k p) n -> p k n", p=128))
        wo_sb = sp.tile([128, 4, C], BF16, name="wo_sb")
        nc.gpsimd.dma_start(wo_sb[:], wo.rearrange("(h p) n -> p h n", p=128))

        kc_t = [sp.tile([128, 4, TB], BF16, name=f"kc{t}") for t in range(NTB)]
        kr_t = [sp.tile([128, 2, TB], BF16, name=f"kr{t}") for t in range(NTB)]
        v_t = [sp.tile([128, 4, TB], BF16, name=f"v{t}") for t in range(NTB)]

        def load_xblk(t):
            xb = sp.tile([128, 16, TB], BF16, name="xblk", tag="xblk", bufs=2)
            tc0 = TB * t
            nc.sync.dma_start(
                xb[:], xT[:, tc0:tc0 + TB].rearrange("(k p) n -> p k n", p=128))
            return xb

        def rope_store(ps_t, dst, cs, sn):
            # ps_t [128, TB] PSUM: rows [64 head 2p | 64 head 2p+1] rope dims
            r16 = sp.tile([128, TB], BF16, name="r16", tag="r16", bufs=2)
            nc.scalar.copy(r16[:], ps_t[:])
            t1 = sp.tile([128, TB], BF16, name="rp1", tag="rp1", bufs=2)
            nc.vector.tensor_mul(t1[:], r16[:], cs)
            sh = sp.tile([128, TB], BF16, name="rp2", tag="rp2", bufs=2)
            nc.vector.tensor_copy(sh[0:32, :], r16[32:64, :])
            nc.vector.tensor_copy(sh[32:64, :], r16[0:32, :])
            nc.vector.tensor_copy(sh[64:96, :], r16[96:128, :])
            nc.vector.tensor_copy(sh[96:128, :], r16[64:96, :])
            nc.vector.tensor_mul(sh[:], sh[:], sn)
            nc.vector.tensor_add(dst, t1[:], sh[:])

        def proj_ckv(t, xb):
            ckvb = sp.tile([128, 4, TB], BF16, name="ckv_blk", tag="ckv", bufs=2)
            for m in range(4):
                ps_t = pp.tile([128, TB], F32, name="ps_p", tag="ps", bufs=4)
                for k in range(16):
                    nc.tensor.matmul(ps_t[:], wdkv_sb[:, k, 128 * m:128 * (m + 1)],
                                     xb[:, k, :], start=(k == 0), stop=(k == 15))
                eng = nc.scalar if m % 2 == 0 else nc.vector
                ecopy(eng, ckvb[:, m, :], ps_t[:])
            return ckvb

        def proj_rest(t, xb, ckvb):
            tc0 = TB * t
            cs = cos_sb[:, tc0:tc0 + TB]
            sn = sin_sb[:, tc0:tc0 + TB]
            qcb = sp.tile([128, 4, TB], BF16, name="qc_blk", tag="qc", bufs=2)
            for h in range(4):
                ps_t = pp.tile([128, TB], F32, name="ps_p", tag="ps", bufs=4)
                for k in range(16):
                    nc.tensor.matmul(
                        ps_t[:], aq_sb[:, k, 128 * h:128 * (h + 1)],
                        xb[:, k, :], start=(k == 0), stop=(k == 15))
                eng = nc.scalar if h % 2 == 0 else nc.vector
                ecopy(eng, qcb[:, h, :], ps_t[:])
            qrb = sp.tile([128, 2, TB], BF16, name="qr_blk", tag="qr", bufs=2)
            for p in range(2):
                ps_t = pp.tile([128, TB], F32, name="ps_p", tag="ps", bufs=4)
                for k in range(16):
                    nc.tensor.matmul(
                        ps_t[:], aqr_sb[:, k, 128 * p:128 * (p + 1)],
                        xb[:, k, :], start=(k == 0), stop=(k == 15))
                rope_store(ps_t, qrb[:, p, :], cs, sn)
            for h in range(4):
                ps_t = pp.tile([128, TB], F32, name="ps_p", tag="ps", bufs=4)
                for k in range(4):
                    nc.tensor.matmul(
                        ps_t[:], wuk_sb[:, k, 128 * h:128 * (h + 1)],
                        ckvb[:, k, :], start=(k == 0), stop=(k == 3))
                eng = nc.scalar if h % 2 == 0 else nc.vector
                ecopy(eng, kc_t[t][:, h, :], ps_t[:])
            for p in range(2):
                ps_t = pp.tile([128, TB], F32, name="ps_p", tag="ps", bufs=4)
                for k in range(4):
                    nc.tensor.matmul(
                        ps_t[:], wkr_sb[:, k, 128 * p:128 * (p + 1)],
                        ckvb[:, k, :], start=(k == 0), stop=(k == 3))
                rope_store(ps_t, kr_t[t][:, p, :], cs, sn)
            for tkc in range(4):
                ps_t = pp.tile([128, TB], F32, name="ps_p", tag="ps", bufs=4)
                for k in range(4):
                    nc.tensor.matmul(
                        ps_t[:], ckvb[:, k, 128 * tkc:128 * (tkc + 1)],
                        wuv_sb[:, k, :], start=(k == 0), stop=(k == 3))
                eng = nc.scalar if tkc % 2 == 0 else nc.vector
                ecopy(eng, v_t[t][:, tkc, :], ps_t[:])
            return qcb, qrb

        def emit_norm(info, avn):
            # recip-broadcast matmul + avn write for a finished head;
            # called once dense PE work sits between it and rec16's producer
            h, ps_av, rec16 = info
            ps_bc = pp.tile([128, TB], F32, name="ps_bc", tag="ps", bufs=4)
            nc.tensor.matmul(ps_bc[:], ones_sb[0:1, :], rec16[:],
                             start=True, stop=True)
            av16 = sp.tile([128, TB], BF16, name="av16", tag="av16", bufs=2)
            nc.scalar.copy(av16[:], ps_av[:])
            nc.vector.tensor_mul(avn[:, TB * h:TB * (h + 1)], av16[:], ps_bc[:])

        def attend(t, qcb, qrb):
            # head pairs (0,1) and (2,3) in lockstep; the two rope matmuls of
            # a pair are adjacent with disjoint row groups (0-63 / 64-127) so
            # the PE runs them concurrently, amortizing the per-MM fixed cost
            nch = 4 * (t + 1)
            avn = sp.tile([128, 4 * TB], BF16, name="avn", tag="avn", bufs=2)
            pending = []
            for pair in range(2):
                heads = (2 * pair, 2 * pair + 1)
                pr = pair

                def qk_pair(c):
                    j = c - 4 * t
                    s = 128 * j if j > 0 else 0
                    blk, jj = divmod(c, 4)
                    res = []
                    for h in heads:
                        ps_s = pp.tile([128, TB], F32, name="ps_s", tag="ps",
                                       bufs=4)
                        nc.tensor.matmul(
                            ps_s[:, s:], kc_t[blk][:, h, 128 * jj:128 * (jj + 1)],
                            qcb[:, h, s:], start=True, stop=False)
                        res.append(ps_s)
                    for ps_s, h in zip(res, heads):
                        p0 = 64 * (h % 2)
                        nc.tensor.matmul(
                            ps_s[:, s:],
                            kr_t[blk][p0:p0 + 64, pr, 128 * jj:128 * (jj + 1)],
                            qrb[p0:p0 + 64, pr, s:], start=False, stop=True)
                    return res

                q = [qk_pair(0)]
                if nch > 1:
                    q.append(qk_pair(1))
                ps_av, ps_sum, dacc = {}, {}, {}
                pendl = {h: [] for h in heads}
                pend_quad = {h: None for h in heads}
                sum_started = {h: False for h in heads}
                for c in range(nch):
                    curpair = q.pop(0)
                    j = c - 4 * t
                    s = 128 * j if j > 0 else 0
                    blk, jj = divmod(c, 4)
                    exs_pair = []
                    for idx, h in enumerate(heads):
                        ex = sp.tile([128, TB], BF16, name="ex", tag="ex",
                                     bufs=10)
                        nc.scalar.activation(ex[:, s:], curpair[idx][:, s:],
                                             AF.Exp, scale=SCALE)
                        if j >= 0:
                            nc.vector.tensor_mul(ex[:, s:s + 128],
                                                 ex[:, s:s + 128], mask_sb[:])
                        exs_pair.append(ex)
                    if c + 2 < nch:
                        q.append(qk_pair(c + 2))
                    if c == 0:
                        for h in heads:
                            ps_av[h] = pp.tile([128, TB], F32, name="ps_av",
                                               tag="ps_av", bufs=2)
                            ps_sum[h] = pp.tile([1, TB], F32, name="ps_sum",
                                                tag="ps_sum", bufs=2)
                        for info in pending:
                            emit_norm(info, avn)
                        pending = []
                    for idx, h in enumerate(heads):
                        nc.tensor.matmul(
                            ps_av[h][:, s:],
                            v_t[blk][:, jj, 128 * h:128 * (h + 1)],
                            exs_pair[idx][:, s:],
                            start=(c == 0), stop=(c == nch - 1))
                    for h in heads:
                        if pend_quad[h] is not None:
                            nc.tensor.matmul(ps_sum[h][:], ones_sb[:, 0:1],
                                             pend_quad[h][:],
                                             start=not sum_started[h],
                                             stop=False)
                            sum_started[h] = True
                            pend_quad[h] = None
                    for idx, h in enumerate(heads):
                        ex = exs_pair[idx]
                        if j < 0:
                            # full-width chunks: pre-add quads on DVE so one
                            # ones-matmul covers four (count 4t -> exact)
                            pendl[h].append(ex)
                            if len(pendl[h]) == 4:
                                exs = sp.tile([128, TB], BF16, name="exs",
                                              tag="exs", bufs=3)
                                nc.vector.tensor_add(exs[:], pendl[h][0][:],
                                                     pendl[h][1][:])
                                nc.vector.tensor_add(exs[:], exs[:],
                                                     pendl[h][2][:])
                                nc.vector.tensor_add(exs[:], exs[:],
                                                     pendl[h][3][:])
                                pend_quad[h] = exs
                                pendl[h] = []
                        else:
                            # diagonal-region chunks: DVE-accumulate so one
                            # ones-matmul covers all four
                            if j == 0:
                                dacc[h] = sp.tile([128, TB], BF16, name="dacc",
                                                  tag="dacc", bufs=2)
                                nc.vector.tensor_copy(dacc[h][:], ex[:])
                            else:
                                nc.vector.tensor_add(dacc[h][:, s:],
                                                     dacc[h][:, s:], ex[:, s:])
                            if c == nch - 1:
                                nc.tensor.matmul(ps_sum[h][:], ones_sb[:, 0:1],
                                                 dacc[h][:],
                                                 start=not sum_started[h],
                                                 stop=True)
                                sum_started[h] = True
                                recip = sp.tile([1, TB], F32, name="recip",
                                                tag="recip", bufs=2)
                                nc.vector.reciprocal(recip[:], ps_sum[h][:])
                                rec16 = sp.tile([1, TB], BF16, name="rec16",
                                                tag="rec16", bufs=2)
                                nc.vector.tensor_copy(rec16[:], recip[:])
                                pending.append((h, ps_av[h], rec16))
            return avn, pending

        def emit_wo(t, avn):
            tc0 = TB * t
            for tqc in range(4):
                for n in range(4):
                    ps_o = pp.tile([128, 512], F32, name="ps_o", tag="ps",
                                   bufs=4)
                    for h in range(4):
                        nc.tensor.matmul(
                            ps_o[:],
                            avn[:, TB * h + 128 * tqc:TB * h + 128 * (tqc + 1)],
                            wo_sb[:, h, 512 * n:512 * (n + 1)],
                            start=(h == 0), stop=(h == 3))
                    ost = sp.tile([128, 512], BF16, name="ost", tag="ost",
                                  bufs=3)
                    eng = nc.scalar if n % 2 == 0 else nc.vector
                    ecopy(eng, ost[:], ps_o[:])
                    nc.sync.dma_start(
                        out[tc0 + 128 * tqc:tc0 + 128 * (tqc + 1),
                            512 * n:512 * (n + 1)], ost[:])

        # ---- body schedule ----
        xb = load_xblk(0)
        ckvb = proj_ckv(0, xb)
        qcb, qrb = proj_rest(0, xb, ckvb)
        for t in range(NTB):
            if t + 1 < NTB:
                xb_next = load_xblk(t + 1)
            avn, pending = attend(t, qcb, qrb)
            if t + 1 < NTB:
                ckvb = proj_ckv(t + 1, xb_next)
            for info in pending:
                emit_norm(info, avn)
            emit_wo(t, avn)
            if t + 1 < NTB:
                qcb, qrb = proj_rest(t + 1, xb_next, ckvb)


def _rope_tables():
    import ml_dtypes
    inv = 1.0 / (ROPE_BASE ** (np.arange(0, R, 2, dtype=np.float32) / R))
    freqs = np.arange(T, dtype=np.float32)[:, None] * inv[None, :]       # [T, 32]
    emb = np.concatenate([freqs, freqs], axis=-1)                         # [T, 64]
    cosT = np.ascontiguousarray(np.cos(emb).T.astype(np.float32))         # [64, T]
    sinT = np.ascontiguousarray(np.sin(emb).T.astype(np.float32))
    cosd = np.concatenate([cosT, cosT], axis=0)                           # [128, T]
    sin_sgn = np.concatenate([-sinT[0:32], sinT[32:64]], axis=0)          # [64, T]
    sind = np.concatenate([sin_sgn, sin_sgn], axis=0)
    return (cosd.astype(ml_dtypes.bfloat16), sind.astype(ml_dtypes.bfloat16))


def host_inmaps(inputs):
    import ml_dtypes
    BF = ml_dtypes.bfloat16

    x = np.asarray(inputs["x"], dtype=np.float32)
    W_dq = np.asarray(inputs["W_dq"], dtype=np.float32)
    W_uq = np.asarray(inputs["W_uq"], dtype=np.float32)
    W_qr = np.asarray(inputs["W_qr"], dtype=np.float32)
    W_dkv = np.asarray(inputs["W_dkv"], dtype=np.float32).astype(BF)
    W_uk = np.asarray(inputs["W_uk"], dtype=np.float32).astype(BF)
    W_uv = np.asarray(inputs["W_uv"], dtype=np.float32).astype(BF)
    W_kr = np.asarray(inputs["W_kr"], dtype=np.float32).astype(BF)
    W_o = np.asarray(inputs["W_o"], dtype=np.float32).astype(BF)

    Aq = (W_dq @ W_uq).astype(BF)     # [C, NH*DH] folded q-content projection
    Aqr = (W_dq @ W_qr).astype(BF)    # [C, NH*R] folded q-rope projection

    cosd, sind = _rope_tables()
    maskv = (np.arange(128)[:, None] <= np.arange(128)[None, :]).astype(
        np.float32).astype(BF)
    onesv = np.ones((128, 128), dtype=np.float32).astype(BF)

    in_maps = []
    for core in range(8):
        b, hg = core // 4, core % 4
        in_maps.append({
            "xT": np.ascontiguousarray(x[b].T).astype(BF),
            "aq": np.ascontiguousarray(Aq[:, 512 * hg:512 * (hg + 1)]),
            "aqr": np.ascontiguousarray(Aqr[:, 256 * hg:256 * (hg + 1)]),
            "wdkv": W_dkv,
            "wuk": np.ascontiguousarray(W_uk[:, 512 * hg:512 * (hg + 1)]),
            "wuv": np.ascontiguousarray(W_uv[:, 512 * hg:512 * (hg + 1)]),
            "wkr": np.ascontiguousarray(W_kr[:, 256 * hg:256 * (hg + 1)]),
            "wo": np.ascontiguousarray(W_o[512 * hg:512 * (hg + 1), :]),
            "cosd": cosd,
            "sind": sind,
            "maskd": maskv,
            "onesd": onesv,
        })
    return in_maps


def kernel(**inputs):
    from concourse.bass_utils import run_bass_kernel_spmd

    if "nc" not in _CACHE:
        _CACHE["nc"] = _build_nc()
    nc = _CACHE["nc"]

    in_maps = host_inmaps(inputs)

    res = run_bass_kernel_spmd(nc, in_maps, core_ids=list(range(8)))
    outs = [np.asarray(r["out"]).astype(np.float32) for r in res.results]
    out0 = outs[0] + outs[1] + outs[2] + outs[3]
    out1 = outs[4] + outs[5] + outs[6] + outs[7]
    return np.stack([out0, out1]).astype(np.float32)


# revision 18
# speedup vs baseline: 1.0975x; 1.0185x over previous
"""Multi-Head Latent Attention (MLA) Bass kernel for 8 trn2 NeuronCores.

Sharding: core c handles batch b=c//4 and head group hg=c%4 (4 of 16 heads).
Host transposes x[b] once and pre-casts everything to bf16; the device
pipeline runs in "transposed" layout (feature dims on SBUF partitions).

v5 design. Measured reality on this platform (see mmbench3): every
matmul costs ~344ns fixed + 0.39ns/output-col with NO pipelining between
consecutive MMs, so MM instruction count dominates; but matmuls whose
row groups are disjoint (K<=64 at partition offsets 0/64) DO execute
concurrently in the PE sub-arrays. Hence:
  - q-path FOLDED on host (A_h = W_dq @ W_uq_h); kv-path two-stage.
  - attention runs head PAIRS in lockstep; the pair's two rope-score
    matmuls (K=64, rows 0-63 / 64-127) are emitted adjacently and run
    concurrently, halving their fixed cost (~36us/body).
  - 1-chunk-pair qk lookahead keeps exp() latency off the PE path.
  - denominator: quad DVE pre-adds (one ones-matmul per 4 full chunks)
    + DVE-merged diagonal chunks (one ones-matmul for all 4, -48 MMs).
  - one shared 4-bank PSUM rotation ("ps") for scores / projections /
    recip-broadcasts / W_o + 2 av banks + 2 sum banks = 8 banks.
  - per-head normalization (recip -> broadcast matmul -> avn mul)
    deferred into the next pair's prologue, off the PE critical path.
  - block order: att(t) -> ckv(t+1) -> norms -> W_o(t) -> proj rest(t+1);
    W_o contracts h=3 last so it starts before avn(3) lands.
  - rope: ACT copy PSUM->SBUF, DVE rotate-half shuffles + muls, bf16
    cos/sin tables.
  - bf16 out (host upcasts+sums); x/out on SP DMA queue, weights split
    over scalar/gpsimd queues (Pool strict-FIFO is a trap for bulk DMA).
"""

import numpy as np

T = 2048
C = 2048
QC = 1536
KV = 512
NH = 16
DH = 128
R = 64
TB = 512           # time block / q-group width
NTB = T // TB      # 4
SCALE = 1.0 / float(np.sqrt(DH + R))
ROPE_BASE = 10000.0

_CACHE = {}


def _build_nc(repeat=1):
    import concourse.bacc as bacc
    import concourse.mybir as mybir
    import concourse.tile as tile

    BF16 = mybir.dt.bfloat16

    nc = bacc.Bacc("TRN2", target_bir_lowering=False, debug=False)

    xT = nc.dram_tensor("xT", [C, T], BF16, kind="ExternalInput")
    aq = nc.dram_tensor("aq", [C, 512], BF16, kind="ExternalInput")
    aqr = nc.dram_tensor("aqr", [C, 256], BF16, kind="ExternalInput")
    wdkv = nc.dram_tensor("wdkv", [C, 128], BF16, kind="ExternalInput")
    wuk = nc.dram_tensor("wuk", [KV, 512], BF16, kind="ExternalInput")
    wuv = nc.dram_tensor("wuv", [KV, 512], BF16, kind="ExternalInput")
    wkr = nc.dram_tensor("wkr", [KV, 256], BF16, kind="ExternalInput")
    wo = nc.dram_tensor("wo", [512, C], BF16, kind="ExternalInput")
    cosd = nc.dram_tensor("cosd", [128, T], BF16, kind="ExternalInput")
    sind = nc.dram_tensor("sind", [128, T], BF16, kind="ExternalInput")
    maskd = nc.dram_tensor("maskd", [128, 128], BF16, kind="ExternalInput")
    onesd = nc.dram_tensor("onesd", [128, 128], BF16, kind="ExternalInput")
    out = nc.dram_tensor("out", [T, C], BF16, kind="ExternalOutput")

    with tile.TileContext(nc) as tc:
        for _rep in range(repeat):
            _emit_body(nc, tc, mybir,
                       xT, aq, aqr, wdkv, wuk, wuv, wkr, wo,
                       cosd, sind, maskd, onesd, out)

    nc.compile()
    return nc


def _emit_body(nc, tc, mybir,
               xT, aq, aqr, wdkv, wuk, wuv, wkr, wo,
               cosd, sind, maskd, onesd, out):
    BF16 = mybir.dt.bfloat16
    F32 = mybir.dt.float32
    AF = mybir.ActivationFunctionType

    def ecopy(eng, dst, src_):
        (eng.copy if eng is nc.scalar else eng.tensor_copy)(dst, src_)

    with (
        tc.tile_pool(name="p1", bufs=1) as sp,
        tc.tile_pool(name="p1ps", bufs=1, space="PSUM") as pp,
        tc.tile_pool(name="p1dram", bufs=2, space="DRAM") as dp,
    ):
        # ---- constant / weight loads, spread over 4 DGE queues ----
        cos_sb = sp.tile([128, T], BF16, name="cos_sb")
        nc.sync.dma_start(cos_sb[:], cosd[:])
        sin_sb = sp.tile([128, T], BF16, name="sin_sb")
        nc.sync.dma_start(sin_sb[:], sind[:])
        mask_sb = sp.tile([128, 128], BF16, name="mask_sb")
        nc.sync.dma_start(mask_sb[:], maskd[:])
        ones_sb = sp.tile([128, 128], BF16, name="ones_sb")
        nc.sync.dma_start(ones_sb[:], onesd[:])
        wdkv_sb = sp.tile([128, 16, 128], BF16, name="wdkv_sb")
        nc.scalar.dma_start(wdkv_sb[:], wdkv.rearrange("(k p) n -> p k n", p=128))
        aq_sb = sp.tile([128, 16, 512], BF16, name="aq_sb")
        nc.scalar.dma_start(aq_sb[:], aq.rearrange("(k p) n -> p k n", p=128))
        aqr_sb = sp.tile([128, 16, 256], BF16, name="aqr_sb")
        nc.scalar.dma_start(aqr_sb[:], aqr.rearrange("(k p) n -> p k n", p=128))
        wuk_sb = sp.tile([128, 4, 512], BF16, name="wuk_sb")
        nc.gpsimd.dma_start(wuk_sb[:], wuk.rearrange("(k p) n -> p k n", p=128))
        wuv_sb = sp.tile([128, 4, 512], BF16, name="wuv_sb")
        nc.gpsimd.dma_start(wuv_sb[:], wuv.rearrange("(k p) n -> p k n", p=128))
        wkr_sb = sp.tile([128, 4, 256], BF16, name="wkr_sb")
        nc.gpsimd.dma_start(wkr_sb[:], wkr.rearrange("(k p) n -> p k n", p=128))
        wo_sb = sp.tile([128, 4, C], BF16, name="wo_sb")
        nc.gpsimd.dma_start(wo_sb[:], wo.rearrange("(h p) n -> p h n", p=128))

        kc_t = [sp.tile([128, 4, TB], BF16, name=f"kc{t}") for t in range(NTB)]
        kr_t = [sp.tile([128, 2, TB], BF16, name=f"kr{t}") for t in range(NTB)]
        v_t = [sp.tile([128, 4, TB], BF16, name=f"v{t}") for t in range(NTB)]

        def load_xblk(t):
            xb = sp.tile([128, 16, TB], BF16, name="xblk", tag="xblk", bufs=2)
            tc0 = TB * t
            nc.sync.dma_start(
                xb[:], xT[:, tc0:tc0 + TB].rearrange("(k p) n -> p k n", p=128))
            return xb

        def rope_store(ps_t, dst, cs, sn):
            # ps_t [128, TB] PSUM: rows [64 head 2p | 64 head 2p+1] rope dims
            r16 = sp.tile([128, TB], BF16, name="r16", tag="r16", bufs=2)
            nc.scalar.copy(r16[:], ps_t[:])
            t1 = sp.tile([128, TB], BF16, name="rp1", tag="rp1", bufs=2)
            nc.vector.tensor_mul(t1[:], r16[:], cs)
            sh = sp.tile([128, TB], BF16, name="rp2", tag="rp2", bufs=2)
            nc.vector.tensor_copy(sh[0:32, :], r16[32:64, :])
            nc.vector.tensor_copy(sh[32:64, :], r16[0:32, :])
            nc.vector.tensor_copy(sh[64:96, :], r16[96:128, :])
            nc.vector.tensor_copy(sh[96:128, :], r16[64:96, :])
            nc.vector.tensor_mul(sh[:], sh[:], sn)
            nc.vector.tensor_add(dst, t1[:], sh[:])

        def proj_ckv(t, xb):
            # each core computes only ITS 128-dim quarter of c_kv (the host
            # ships the matching wdkv slice per head-group core), then a
            # 4-core DRAM AllGather reassembles the full [128,4,TB] c_kv in
            # exactly the old layout: ckvb[:, m, :] = dims 128m..128m+127
            ps_t = pp.tile([128, TB], F32, name="ps_p", tag="ps", bufs=4)
            for k in range(16):
                nc.tensor.matmul(ps_t[:], wdkv_sb[:, k, :],
                                 xb[:, k, :], start=(k == 0), stop=(k == 15))
            myq = sp.tile([128, TB], BF16, name="myq", tag="myq", bufs=2)
            nc.scalar.copy(myq[:], ps_t[:])
            cin = dp.tile([128, TB], BF16, name="ccin", tag="ccin", bufs=2)
            nc.gpsimd.dma_start(cin[:], myq[:])
            cout = dp.tile([4, 128, TB], BF16, name="ccout", tag="ccout",
                           bufs=2)
            nc.gpsimd.collective_compute(
                "AllGather", mybir.AluOpType.bypass,
                replica_groups=[[0, 1, 2, 3], [4, 5, 6, 7]],
                ins=[cin.opt()], outs=[cout.opt()])
            ckvb = sp.tile([128, 4, TB], BF16, name="ckv_blk", tag="ckv", bufs=2)
            nc.gpsimd.dma_start(ckvb[:], cout[:].rearrange("c p n -> p c n"))
            return ckvb

        def proj_rest(t, xb, ckvb):
            tc0 = TB * t
            cs = cos_sb[:, tc0:tc0 + TB]
            sn = sin_sb[:, tc0:tc0 + TB]
            qcb = sp.tile([128, 4, TB], BF16, name="qc_blk", tag="qc", bufs=2)
            for h in range(4):
                ps_t = pp.tile([128, TB], F32, name="ps_p", tag="ps", bufs=4)
                for k in range(16):
                    nc.tensor.matmul(
                        ps_t[:], aq_sb[:, k, 128 * h:128 * (h + 1)],
                        xb[:, k, :], start=(k == 0), stop=(k == 15))
                eng = nc.scalar if h % 2 == 0 else nc.vector
                ecopy(eng, qcb[:, h, :], ps_t[:])
            qrb = sp.tile([128, 2, TB], BF16, name="qr_blk", tag="qr", bufs=2)
            for p in range(2):
                ps_t = pp.tile([128, TB], F32, name="ps_p", tag="ps", bufs=4)
                for k in range(16):
                    nc.tensor.matmul(
                        ps_t[:], aqr_sb[:, k, 128 * p:128 * (p + 1)],
                        xb[:, k, :], start=(k == 0), stop=(k == 15))
                rope_store(ps_t, qrb[:, p, :], cs, sn)
            for h in range(4):
                ps_t = pp.tile([128, TB], F32, name="ps_p", tag="ps", bufs=4)
                for k in range(4):
                    nc.tensor.matmul(
                        ps_t[:], wuk_sb[:, k, 128 * h:128 * (h + 1)],
                        ckvb[:, k, :], start=(k == 0), stop=(k == 3))
                eng = nc.scalar if h % 2 == 0 else nc.vector
                ecopy(eng, kc_t[t][:, h, :], ps_t[:])
            for p in range(2):
                ps_t = pp.tile([128, TB], F32, name="ps_p", tag="ps", bufs=4)
                for k in range(4):
                    nc.tensor.matmul(
                        ps_t[:], wkr_sb[:, k, 128 * p:128 * (p + 1)],
                        ckvb[:, k, :], start=(k == 0), stop=(k == 3))
                rope_store(ps_t, kr_t[t][:, p, :], cs, sn)
            for tkc in range(4):
                ps_t = pp.tile([128, TB], F32, name="ps_p", tag="ps", bufs=4)
                for k in range(4):
                    nc.tensor.matmul(
                        ps_t[:], ckvb[:, k, 128 * tkc:128 * (tkc + 1)],
                        wuv_sb[:, k, :], start=(k == 0), stop=(k == 3))
                eng = nc.scalar if tkc % 2 == 0 else nc.vector
                ecopy(eng, v_t[t][:, tkc, :], ps_t[:])
            return qcb, qrb

        def emit_norm(info, avn):
            # recip-broadcast matmul + avn write for a finished head;
            # called once dense PE work sits between it and rec16's producer
            h, ps_av, rec16 = info
            ps_bc = pp.tile([128, TB], F32, name="ps_bc", tag="ps", bufs=4)
            nc.tensor.matmul(ps_bc[:], ones_sb[0:1, :], rec16[:],
                             start=True, stop=True)
            av16 = sp.tile([128, TB], BF16, name="av16", tag="av16", bufs=2)
            nc.scalar.copy(av16[:], ps_av[:])
            nc.vector.tensor_mul(avn[:, TB * h:TB * (h + 1)], av16[:], ps_bc[:])

        def attend(t, qcb, qrb):
            # head pairs (0,1) and (2,3) in lockstep; the two rope matmuls of
            # a pair are adjacent with disjoint row groups (0-63 / 64-127) so
            # the PE runs them concurrently, amortizing the per-MM fixed cost
            nch = 4 * (t + 1)
            avn = sp.tile([128, 4 * TB], BF16, name="avn", tag="avn", bufs=2)
            pending = []
            for pair in range(2):
                heads = (2 * pair, 2 * pair + 1)
                pr = pair

                def qk_pair(c):
                    j = c - 4 * t
                    s = 128 * j if j > 0 else 0
                    blk, jj = divmod(c, 4)
                    res = []
                    for h in heads:
                        ps_s = pp.tile([128, TB], F32, name="ps_s", tag="ps",
                                       bufs=4)
                        nc.tensor.matmul(
                            ps_s[:, s:], kc_t[blk][:, h, 128 * jj:128 * (jj + 1)],
                            qcb[:, h, s:], start=True, stop=False)
                        res.append(ps_s)
                    for ps_s, h in zip(res, heads):
                        p0 = 64 * (h % 2)
                        nc.tensor.matmul(
                            ps_s[:, s:],
                            kr_t[blk][p0:p0 + 64, pr, 128 * jj:128 * (jj + 1)],
                            qrb[p0:p0 + 64, pr, s:], start=False, stop=True)
                    return res

                q = [qk_pair(0)]
                if nch > 1:
                    q.append(qk_pair(1))
                ps_av, ps_sum, dacc = {}, {}, {}
                pendl = {h: [] for h in heads}
                pend_quad = {h: None for h in heads}
                sum_started = {h: False for h in heads}
                for c in range(nch):
                    curpair = q.pop(0)
                    j = c - 4 * t
                    s = 128 * j if j > 0 else 0
                    blk, jj = divmod(c, 4)
                    exs_pair = []
                    for idx, h in enumerate(heads):
                        ex = sp.tile([128, TB], BF16, name="ex", tag="ex",
                                     bufs=10)
                        nc.scalar.activation(ex[:, s:], curpair[idx][:, s:],
                                             AF.Exp, scale=SCALE)
                        if j >= 0:
                            nc.vector.tensor_mul(ex[:, s:s + 128],
                                                 ex[:, s:s + 128], mask_sb[:])
                        exs_pair.append(ex)
                    if c + 2 < nch:
                        q.append(qk_pair(c + 2))
                    if c == 0:
                        for h in heads:
                            ps_av[h] = pp.tile([128, TB], F32, name="ps_av",
                                               tag="ps_av", bufs=2)
                            ps_sum[h] = pp.tile([1, TB], F32, name="ps_sum",
                                                tag="ps_sum", bufs=2)
                        for info in pending:
                            emit_norm(info, avn)
                        pending = []
                    for idx, h in enumerate(heads):
                        nc.tensor.matmul(
                            ps_av[h][:, s:],
                            v_t[blk][:, jj, 128 * h:128 * (h + 1)],
                            exs_pair[idx][:, s:],
                            start=(c == 0), stop=(c == nch - 1))
                    for h in heads:
                        if pend_quad[h] is not None:
                            nc.tensor.matmul(ps_sum[h][:], ones_sb[:, 0:1],
                                             pend_quad[h][:],
                                             start=not sum_started[h],
                                             stop=False)
                            sum_started[h] = True
                            pend_quad[h] = None
                    for idx, h in enumerate(heads):
                        ex = exs_pair[idx]
                        if j < 0:
                            # full-width chunks: pre-add quads on DVE so one
                            # ones-matmul covers four (count 4t -> exact)
                            pendl[h].append(ex)
                            if len(pendl[h]) == 4:
                                exs = sp.tile([128, TB], BF16, name="exs",
                                              tag="exs", bufs=3)
                                nc.vector.tensor_add(exs[:], pendl[h][0][:],
                                                     pendl[h][1][:])
                                nc.vector.tensor_add(exs[:], exs[:],
                                                     pendl[h][2][:])
                                nc.vector.tensor_add(exs[:], exs[:],
                                                     pendl[h][3][:])
                                pend_quad[h] = exs
                                pendl[h] = []
                        else:
                            # diagonal-region chunks: DVE-accumulate so one
                            # ones-matmul covers all four
                            if j == 0:
                                dacc[h] = sp.tile([128, TB], BF16, name="dacc",
                                                  tag="dacc", bufs=2)
                                nc.vector.tensor_copy(dacc[h][:], ex[:])
                            else:
                                nc.vector.tensor_add(dacc[h][:, s:],
                                                     dacc[h][:, s:], ex[:, s:])
                            if c == nch - 1:
                                nc.tensor.matmul(ps_sum[h][:], ones_sb[:, 0:1],
                                                 dacc[h][:],
                                                 start=not sum_started[h],
                                                 stop=True)
                                sum_started[h] = True
                                recip = sp.tile([1, TB], F32, name="recip",
                                                tag="recip", bufs=2)
                                nc.vector.reciprocal(recip[:], ps_sum[h][:])
                                rec16 = sp.tile([1, TB], BF16, name="rec16",
                                                tag="rec16", bufs=2)
                                nc.vector.tensor_copy(rec16[:], recip[:])
                                pending.append((h, ps_av[h], rec16))
            return avn, pending

        def emit_wo(t, avn):
            tc0 = TB * t
            for tqc in range(4):
                for n in range(4):
                    ps_o = pp.tile([128, 512], F32, name="ps_o", tag="ps",
                                   bufs=4)
                    for h in range(4):
                        nc.tensor.matmul(
                            ps_o[:],
                            avn[:, TB * h + 128 * tqc:TB * h + 128 * (tqc + 1)],
                            wo_sb[:, h, 512 * n:512 * (n + 1)],
                            start=(h == 0), stop=(h == 3))
                    ost = sp.tile([128, 512], BF16, name="ost", tag="ost",
                                  bufs=3)
                    eng = nc.scalar if n % 2 == 0 else nc.vector
                    ecopy(eng, ost[:], ps_o[:])
                    nc.sync.dma_start(
                        out[tc0 + 128 * tqc:tc0 + 128 * (tqc + 1),
                            512 * n:512 * (n + 1)], ost[:])

        # ---- body schedule ----
        xb = load_xblk(0)
        ckvb = proj_ckv(0, xb)
        qcb, qrb = proj_rest(0, xb, ckvb)
        for t in range(NTB):
            if t + 1 < NTB:
                xb_next = load_xblk(t + 1)
            avn, pending = attend(t, qcb, qrb)
            if t + 1 < NTB:
                ckvb = proj_ckv(t + 1, xb_next)
            for info in pending:
                emit_norm(info, avn)
            emit_wo(t, avn)
            if t + 1 < NTB:
                qcb, qrb = proj_rest(t + 1, xb_next, ckvb)


def _rope_tables():
    import ml_dtypes
    inv = 1.0 / (ROPE_BASE ** (np.arange(0, R, 2, dtype=np.float32) / R))
    freqs = np.arange(T, dtype=np.float32)[:, None] * inv[None, :]       # [T, 32]
    emb = np.concatenate([freqs, freqs], axis=-1)                         # [T, 64]
    cosT = np.ascontiguousarray(np.cos(emb).T.astype(np.float32))         # [64, T]
    sinT = np.ascontiguousarray(np.sin(emb).T.astype(np.float32))
    cosd = np.concatenate([cosT, cosT], axis=0)                           # [128, T]
    sin_sgn = np.concatenate([-sinT[0:32], sinT[32:64]], axis=0)          # [64, T]
    sind = np.concatenate([sin_sgn, sin_sgn], axis=0)
    return (cosd.astype(ml_dtypes.bfloat16), sind.astype(ml_dtypes.bfloat16))


def host_inmaps(inputs):
    import ml_dtypes
    BF = ml_dtypes.bfloat16

    x = np.asarray(inputs["x"], dtype=np.float32)
    W_dq = np.asarray(inputs["W_dq"], dtype=np.float32)
    W_uq = np.asarray(inputs["W_uq"], dtype=np.float32)
    W_qr = np.asarray(inputs["W_qr"], dtype=np.float32)
    W_dkv = np.asarray(inputs["W_dkv"], dtype=np.float32).astype(BF)
    W_uk = np.asarray(inputs["W_uk"], dtype=np.float32).astype(BF)
    W_uv = np.asarray(inputs["W_uv"], dtype=np.float32).astype(BF)
    W_kr = np.asarray(inputs["W_kr"], dtype=np.float32).astype(BF)
    W_o = np.asarray(inputs["W_o"], dtype=np.float32).astype(BF)

    Aq = (W_dq @ W_uq).astype(BF)     # [C, NH*DH] folded q-content projection
    Aqr = (W_dq @ W_qr).astype(BF)    # [C, NH*R] folded q-rope projection

    cosd, sind = _rope_tables()
    maskv = (np.arange(128)[:, None] <= np.arange(128)[None, :]).astype(
        np.float32).astype(BF)
    onesv = np.ones((128, 128), dtype=np.float32).astype(BF)

    in_maps = []
    for core in range(8):
        b, hg = core // 4, core % 4
        in_maps.append({
            "xT": np.ascontiguousarray(x[b].T).astype(BF),
            "aq": np.ascontiguousarray(Aq[:, 512 * hg:512 * (hg + 1)]),
            "aqr": np.ascontiguousarray(Aqr[:, 256 * hg:256 * (hg + 1)]),
            "wdkv": np.ascontiguousarray(W_dkv[:, 128 * hg:128 * (hg + 1)]),
            "wuk": np.ascontiguousarray(W_uk[:, 512 * hg:512 * (hg + 1)]),
            "wuv": np.ascontiguousarray(W_uv[:, 512 * hg:512 * (hg + 1)]),
            "wkr": np.ascontiguousarray(W_kr[:, 256 * hg:256 * (hg + 1)]),
            "wo": np.ascontiguousarray(W_o[512 * hg:512 * (hg + 1), :]),
            "cosd": cosd,
            "sind": sind,
            "maskd": maskv,
            "onesd": onesv,
        })
    return in_maps


def kernel(**inputs):
    from concourse.bass_utils import run_bass_kernel_spmd

    if "nc" not in _CACHE:
        _CACHE["nc"] = _build_nc()
    nc = _CACHE["nc"]

    in_maps = host_inmaps(inputs)

    res = run_bass_kernel_spmd(nc, in_maps, core_ids=list(range(8)))
    outs = [np.asarray(r["out"]).astype(np.float32) for r in res.results]
    out0 = outs[0] + outs[1] + outs[2] + outs[3]
    out1 = outs[4] + outs[5] + outs[6] + outs[7]
    return np.stack([out0, out1]).astype(np.float32)


# revision 21
# speedup vs baseline: 1.1388x; 1.0376x over previous
"""Multi-Head Latent Attention (MLA) Bass kernel for 8 trn2 NeuronCores.

Sharding: core c handles batch b=c//4 and head group hg=c%4 (4 of 16 heads).
Host transposes x[b] once and pre-casts everything to bf16; the device
pipeline runs in "transposed" layout (feature dims on SBUF partitions).

v5 design. Measured reality on this platform (see mmbench3): every
matmul costs ~344ns fixed + 0.39ns/output-col with NO pipelining between
consecutive MMs, so MM instruction count dominates; but matmuls whose
row groups are disjoint (K<=64 at partition offsets 0/64) DO execute
concurrently in the PE sub-arrays. Hence:
  - q-path FOLDED on host (A_h = W_dq @ W_uq_h); kv-path two-stage.
  - attention runs head PAIRS in lockstep; the pair's two rope-score
    matmuls (K=64, rows 0-63 / 64-127) are emitted adjacently and run
    concurrently, halving their fixed cost (~36us/body).
  - 1-chunk-pair qk lookahead keeps exp() latency off the PE path.
  - denominator: quad DVE pre-adds (one ones-matmul per 4 full chunks)
    + DVE-merged diagonal chunks (one ones-matmul for all 4, -48 MMs).
  - one shared 4-bank PSUM rotation ("ps") for scores / projections /
    recip-broadcasts / W_o + 2 av banks + 2 sum banks = 8 banks.
  - per-head normalization (recip -> broadcast matmul -> avn mul)
    deferred into the next pair's prologue, off the PE critical path.
  - block order: att(t) -> ckv(t+1) -> norms -> W_o(t) -> proj rest(t+1);
    W_o contracts h=3 last so it starts before avn(3) lands.
  - rope: ACT copy PSUM->SBUF, DVE rotate-half shuffles + muls, bf16
    cos/sin tables.
  - bf16 out (host upcasts+sums); x/out on SP DMA queue, weights split
    over scalar/gpsimd queues (Pool strict-FIFO is a trap for bulk DMA).
"""

import numpy as np

T = 2048
C = 2048
QC = 1536
KV = 512
NH = 16
DH = 128
R = 64
TB = 512           # time block / q-group width
NTB = T // TB      # 4
SCALE = 1.0 / float(np.sqrt(DH + R))
ROPE_BASE = 10000.0

_CACHE = {}


def _build_nc(repeat=1):
    import concourse.bacc as bacc
    import concourse.mybir as mybir
    import concourse.tile as tile

    BF16 = mybir.dt.bfloat16

    nc = bacc.Bacc("TRN2", target_bir_lowering=False, debug=False)

    xT = nc.dram_tensor("xT", [C, T], BF16, kind="ExternalInput")
    aq = nc.dram_tensor("aq", [C, 512], BF16, kind="ExternalInput")
    aqr = nc.dram_tensor("aqr", [C, 256], BF16, kind="ExternalInput")
    wdkv = nc.dram_tensor("wdkv", [C, 128], BF16, kind="ExternalInput")
    wuk = nc.dram_tensor("wuk", [KV, 512], BF16, kind="ExternalInput")
    wuv = nc.dram_tensor("wuv", [KV, 512], BF16, kind="ExternalInput")
    wkr = nc.dram_tensor("wkr", [KV, 256], BF16, kind="ExternalInput")
    wo = nc.dram_tensor("wo", [512, C], BF16, kind="ExternalInput")
    cosd = nc.dram_tensor("cosd", [128, T], BF16, kind="ExternalInput")
    sind = nc.dram_tensor("sind", [128, T], BF16, kind="ExternalInput")
    maskd = nc.dram_tensor("maskd", [128, 128], BF16, kind="ExternalInput")
    onesd = nc.dram_tensor("onesd", [128, 128], BF16, kind="ExternalInput")
    out = nc.dram_tensor("out", [T, C], BF16, kind="ExternalOutput")

    with tile.TileContext(nc) as tc:
        for _rep in range(repeat):
            _emit_body(nc, tc, mybir,
                       xT, aq, aqr, wdkv, wuk, wuv, wkr, wo,
                       cosd, sind, maskd, onesd, out)

    nc.compile()
    return nc


def _emit_body(nc, tc, mybir,
               xT, aq, aqr, wdkv, wuk, wuv, wkr, wo,
               cosd, sind, maskd, onesd, out):
    BF16 = mybir.dt.bfloat16
    F32 = mybir.dt.float32
    AF = mybir.ActivationFunctionType

    def ecopy(eng, dst, src_):
        (eng.copy if eng is nc.scalar else eng.tensor_copy)(dst, src_)

    with (
        tc.tile_pool(name="p1", bufs=1) as sp,
        tc.tile_pool(name="p1ps", bufs=1, space="PSUM") as pp,
        tc.tile_pool(name="p1dram", bufs=2, space="DRAM") as dp,
    ):
        # ---- constant / weight loads, spread over 4 DGE queues ----
        cos_sb = sp.tile([128, T], BF16, name="cos_sb")
        nc.sync.dma_start(cos_sb[:], cosd[:])
        sin_sb = sp.tile([128, T], BF16, name="sin_sb")
        nc.sync.dma_start(sin_sb[:], sind[:])
        mask_sb = sp.tile([128, 128], BF16, name="mask_sb")
        nc.sync.dma_start(mask_sb[:], maskd[:])
        ones_sb = sp.tile([128, 128], BF16, name="ones_sb")
        nc.sync.dma_start(ones_sb[:], onesd[:])
        wdkv_sb = sp.tile([128, 16, 128], BF16, name="wdkv_sb")
        nc.scalar.dma_start(wdkv_sb[:], wdkv.rearrange("(k p) n -> p k n", p=128))
        aq_sb = sp.tile([128, 16, 512], BF16, name="aq_sb")
        nc.scalar.dma_start(aq_sb[:], aq.rearrange("(k p) n -> p k n", p=128))
        aqr_sb = sp.tile([128, 16, 256], BF16, name="aqr_sb")
        nc.scalar.dma_start(aqr_sb[:], aqr.rearrange("(k p) n -> p k n", p=128))
        wuk_sb = sp.tile([128, 4, 512], BF16, name="wuk_sb")
        nc.gpsimd.dma_start(wuk_sb[:], wuk.rearrange("(k p) n -> p k n", p=128))
        wuv_sb = sp.tile([128, 4, 512], BF16, name="wuv_sb")
        nc.gpsimd.dma_start(wuv_sb[:], wuv.rearrange("(k p) n -> p k n", p=128))
        wkr_sb = sp.tile([128, 4, 256], BF16, name="wkr_sb")
        nc.gpsimd.dma_start(wkr_sb[:], wkr.rearrange("(k p) n -> p k n", p=128))
        wo_sb = sp.tile([128, 4, C], BF16, name="wo_sb")
        nc.gpsimd.dma_start(wo_sb[:], wo.rearrange("(h p) n -> p h n", p=128))

        kc_t = [sp.tile([128, 4, TB], BF16, name=f"kc{t}") for t in range(NTB)]
        kr_t = [sp.tile([128, 2, TB], BF16, name=f"kr{t}") for t in range(NTB)]
        v_t = [sp.tile([128, 4, TB], BF16, name=f"v{t}") for t in range(NTB)]

        def load_xblk(t):
            xb = sp.tile([128, 16, TB], BF16, name="xblk", tag="xblk", bufs=2)
            tc0 = TB * t
            nc.sync.dma_start(
                xb[:], xT[:, tc0:tc0 + TB].rearrange("(k p) n -> p k n", p=128))
            return xb

        def rope_store(ps_t, dst, cs, sn):
            # ps_t [128, TB] PSUM: rows [64 head 2p | 64 head 2p+1] rope dims
            r16 = sp.tile([128, TB], BF16, name="r16", tag="r16", bufs=2)
            nc.scalar.copy(r16[:], ps_t[:])
            t1 = sp.tile([128, TB], BF16, name="rp1", tag="rp1", bufs=2)
            nc.vector.tensor_mul(t1[:], r16[:], cs)
            sh = sp.tile([128, TB], BF16, name="rp2", tag="rp2", bufs=2)
            nc.vector.tensor_copy(sh[0:32, :], r16[32:64, :])
            nc.vector.tensor_copy(sh[32:64, :], r16[0:32, :])
            nc.vector.tensor_copy(sh[64:96, :], r16[96:128, :])
            nc.vector.tensor_copy(sh[96:128, :], r16[64:96, :])
            nc.vector.tensor_mul(sh[:], sh[:], sn)
            nc.vector.tensor_add(dst, t1[:], sh[:])

        def proj_ckv(t, xb):
            # each core computes only ITS 128-dim quarter of c_kv (the host
            # ships the matching wdkv slice per head-group core), then a
            # 4-core DRAM AllGather reassembles the full [128,4,TB] c_kv in
            # exactly the old layout: ckvb[:, m, :] = dims 128m..128m+127
            ps_t = pp.tile([128, TB], F32, name="ps_p", tag="ps", bufs=4)
            for k in range(16):
                nc.tensor.matmul(ps_t[:], wdkv_sb[:, k, :],
                                 xb[:, k, :], start=(k == 0), stop=(k == 15))
            myq = sp.tile([128, TB], BF16, name="myq", tag="myq", bufs=2)
            nc.scalar.copy(myq[:], ps_t[:])
            cin = dp.tile([128, TB], BF16, name="ccin", tag="ccin", bufs=2)
            nc.gpsimd.dma_start(cin[:], myq[:])
            cout = dp.tile([4, 128, TB], BF16, name="ccout", tag="ccout",
                           bufs=2)
            nc.gpsimd.collective_compute(
                "AllGather", mybir.AluOpType.bypass,
                replica_groups=[[0, 1, 2, 3], [4, 5, 6, 7]],
                ins=[cin.opt()], outs=[cout.opt()])
            ckvb = sp.tile([128, 4, TB], BF16, name="ckv_blk", tag="ckv", bufs=2)
            nc.gpsimd.dma_start(ckvb[:], cout[:].rearrange("c p n -> p c n"))
            return ckvb

        def proj_rest(t, xb, ckvb):
            tc0 = TB * t
            cs = cos_sb[:, tc0:tc0 + TB]
            sn = sin_sb[:, tc0:tc0 + TB]
            qcb = sp.tile([128, 4, TB], BF16, name="qc_blk", tag="qc", bufs=2)
            for h in range(4):
                ps_t = pp.tile([128, TB], F32, name="ps_p", tag="ps", bufs=4)
                for k in range(16):
                    nc.tensor.matmul(
                        ps_t[:], aq_sb[:, k, 128 * h:128 * (h + 1)],
                        xb[:, k, :], start=(k == 0), stop=(k == 15))
                eng = nc.scalar if h % 2 == 0 else nc.vector
                ecopy(eng, qcb[:, h, :], ps_t[:])
            qrb = sp.tile([128, 2, TB], BF16, name="qr_blk", tag="qr", bufs=2)
            for p in range(2):
                ps_t = pp.tile([128, TB], F32, name="ps_p", tag="ps", bufs=4)
                for k in range(16):
                    nc.tensor.matmul(
                        ps_t[:], aqr_sb[:, k, 128 * p:128 * (p + 1)],
                        xb[:, k, :], start=(k == 0), stop=(k == 15))
                rope_store(ps_t, qrb[:, p, :], cs, sn)
            for h in range(4):
                ps_t = pp.tile([128, TB], F32, name="ps_p", tag="ps", bufs=4)
                for k in range(4):
                    nc.tensor.matmul(
                        ps_t[:], wuk_sb[:, k, 128 * h:128 * (h + 1)],
                        ckvb[:, k, :], start=(k == 0), stop=(k == 3))
                eng = nc.scalar if h % 2 == 0 else nc.vector
                ecopy(eng, kc_t[t][:, h, :], ps_t[:])
            for p in range(2):
                ps_t = pp.tile([128, TB], F32, name="ps_p", tag="ps", bufs=4)
                for k in range(4):
                    nc.tensor.matmul(
                        ps_t[:], wkr_sb[:, k, 128 * p:128 * (p + 1)],
                        ckvb[:, k, :], start=(k == 0), stop=(k == 3))
                rope_store(ps_t, kr_t[t][:, p, :], cs, sn)
            for tkc in range(4):
                ps_t = pp.tile([128, TB], F32, name="ps_p", tag="ps", bufs=4)
                for k in range(4):
                    nc.tensor.matmul(
                        ps_t[:], ckvb[:, k, 128 * tkc:128 * (tkc + 1)],
                        wuv_sb[:, k, :], start=(k == 0), stop=(k == 3))
                eng = nc.scalar if tkc % 2 == 0 else nc.vector
                ecopy(eng, v_t[t][:, tkc, :], ps_t[:])
            return qcb, qrb

        def emit_norm(info, avn):
            # recip-broadcast matmul + avn write for a finished head;
            # called once dense PE work sits between it and rec16's producer
            h, ps_av, rec16 = info
            ps_bc = pp.tile([128, TB], F32, name="ps_bc", tag="ps", bufs=4)
            nc.tensor.matmul(ps_bc[:], ones_sb[0:1, :], rec16[:],
                             start=True, stop=True)
            av16 = sp.tile([128, TB], BF16, name="av16", tag="av16", bufs=2)
            nc.scalar.copy(av16[:], ps_av[:])
            nc.vector.tensor_mul(avn[:, TB * h:TB * (h + 1)], av16[:], ps_bc[:])

        def attend(t, qcb, qrb, mid=None):
            # head pairs (0,1) and (2,3) in lockstep; the two rope matmuls of
            # a pair are adjacent with disjoint row groups (0-63 / 64-127) so
            # the PE runs them concurrently, amortizing the per-MM fixed cost.
            # `mid` (next block's c_kv quarter + all-gather) is emitted
            # between the pairs so the collective's latency hides under
            # pair 2 + W_o + the q projections.
            nch = 4 * (t + 1)
            avn = sp.tile([128, 4 * TB], BF16, name="avn", tag="avn", bufs=2)
            pending = []
            mid_result = None
            for pair in range(2):
                if pair == 1 and mid is not None:
                    mid_result = mid()
                heads = (2 * pair, 2 * pair + 1)
                pr = pair

                def qk_pair(c):
                    j = c - 4 * t
                    s = 128 * j if j > 0 else 0
                    blk, jj = divmod(c, 4)
                    res = []
                    for h in heads:
                        ps_s = pp.tile([128, TB], F32, name="ps_s", tag="ps",
                                       bufs=4)
                        nc.tensor.matmul(
                            ps_s[:, s:], kc_t[blk][:, h, 128 * jj:128 * (jj + 1)],
                            qcb[:, h, s:], start=True, stop=False)
                        res.append(ps_s)
                    for ps_s, h in zip(res, heads):
                        p0 = 64 * (h % 2)
                        nc.tensor.matmul(
                            ps_s[:, s:],
                            kr_t[blk][p0:p0 + 64, pr, 128 * jj:128 * (jj + 1)],
                            qrb[p0:p0 + 64, pr, s:], start=False, stop=True)
                    return res

                q = [qk_pair(0)]
                if nch > 1:
                    q.append(qk_pair(1))
                ps_av, ps_sum, dacc = {}, {}, {}
                pendl = {h: [] for h in heads}
                pend_quad = {h: None for h in heads}
                sum_started = {h: False for h in heads}
                for c in range(nch):
                    curpair = q.pop(0)
                    j = c - 4 * t
                    s = 128 * j if j > 0 else 0
                    blk, jj = divmod(c, 4)
                    exs_pair = []
                    for idx, h in enumerate(heads):
                        ex = sp.tile([128, TB], BF16, name="ex", tag="ex",
                                     bufs=10)
                        nc.scalar.activation(ex[:, s:], curpair[idx][:, s:],
                                             AF.Exp, scale=SCALE)
                        if j >= 0:
                            nc.vector.tensor_mul(ex[:, s:s + 128],
                                                 ex[:, s:s + 128], mask_sb[:])
                        exs_pair.append(ex)
                    if c + 2 < nch:
                        q.append(qk_pair(c + 2))
                    if c == 0:
                        for h in heads:
                            ps_av[h] = pp.tile([128, TB], F32, name="ps_av",
                                               tag="ps_av", bufs=2)
                            ps_sum[h] = pp.tile([1, TB], F32, name="ps_sum",
                                                tag="ps_sum", bufs=2)
                        for info in pending:
                            emit_norm(info, avn)
                        pending = []
                    for idx, h in enumerate(heads):
                        nc.tensor.matmul(
                            ps_av[h][:, s:],
                            v_t[blk][:, jj, 128 * h:128 * (h + 1)],
                            exs_pair[idx][:, s:],
                            start=(c == 0), stop=(c == nch - 1))
                    for h in heads:
                        if pend_quad[h] is not None:
                            nc.tensor.matmul(ps_sum[h][:], ones_sb[:, 0:1],
                                             pend_quad[h][:],
                                             start=not sum_started[h],
                                             stop=False)
                            sum_started[h] = True
                            pend_quad[h] = None
                    for idx, h in enumerate(heads):
                        ex = exs_pair[idx]
                        if j < 0:
                            # full-width chunks: pre-add quads on DVE so one
                            # ones-matmul covers four (count 4t -> exact)
                            pendl[h].append(ex)
                            if len(pendl[h]) == 4:
                                exs = sp.tile([128, TB], BF16, name="exs",
                                              tag="exs", bufs=3)
                                nc.vector.tensor_add(exs[:], pendl[h][0][:],
                                                     pendl[h][1][:])
                                nc.vector.tensor_add(exs[:], exs[:],
                                                     pendl[h][2][:])
                                nc.vector.tensor_add(exs[:], exs[:],
                                                     pendl[h][3][:])
                                pend_quad[h] = exs
                                pendl[h] = []
                        else:
                            # diagonal-region chunks: DVE-accumulate so one
                            # ones-matmul covers all four
                            if j == 0:
                                dacc[h] = sp.tile([128, TB], BF16, name="dacc",
                                                  tag="dacc", bufs=2)
                                nc.vector.tensor_copy(dacc[h][:], ex[:])
                            else:
                                nc.vector.tensor_add(dacc[h][:, s:],
                                                     dacc[h][:, s:], ex[:, s:])
                            if c == nch - 1:
                                nc.tensor.matmul(ps_sum[h][:], ones_sb[:, 0:1],
                                                 dacc[h][:],
                                                 start=not sum_started[h],
                                                 stop=True)
                                sum_started[h] = True
                                recip = sp.tile([1, TB], F32, name="recip",
                                                tag="recip", bufs=2)
                                nc.vector.reciprocal(recip[:], ps_sum[h][:])
                                rec16 = sp.tile([1, TB], BF16, name="rec16",
                                                tag="rec16", bufs=2)
                                nc.vector.tensor_copy(rec16[:], recip[:])
                                pending.append((h, ps_av[h], rec16))
            return avn, pending, mid_result

        def emit_wo(t, avn):
            tc0 = TB * t
            for tqc in range(4):
                for n in range(4):
                    ps_o = pp.tile([128, 512], F32, name="ps_o", tag="ps",
                                   bufs=4)
                    for h in range(4):
                        nc.tensor.matmul(
                            ps_o[:],
                            avn[:, TB * h + 128 * tqc:TB * h + 128 * (tqc + 1)],
                            wo_sb[:, h, 512 * n:512 * (n + 1)],
                            start=(h == 0), stop=(h == 3))
                    ost = sp.tile([128, 512], BF16, name="ost", tag="ost",
                                  bufs=3)
                    eng = nc.scalar if n % 2 == 0 else nc.vector
                    ecopy(eng, ost[:], ps_o[:])
                    nc.sync.dma_start(
                        out[tc0 + 128 * tqc:tc0 + 128 * (tqc + 1),
                            512 * n:512 * (n + 1)], ost[:])

        # ---- body schedule ----
        xb = load_xblk(0)
        ckvb = proj_ckv(0, xb)
        qcb, qrb = proj_rest(0, xb, ckvb)
        import functools
        for t in range(NTB):
            mid = None
            if t + 1 < NTB:
                xb_next = load_xblk(t + 1)
                mid = functools.partial(proj_ckv, t + 1, xb_next)
            avn, pending, ckvb = attend(t, qcb, qrb, mid=mid)
            for info in pending:
                emit_norm(info, avn)
            emit_wo(t, avn)
            if t + 1 < NTB:
                qcb, qrb = proj_rest(t + 1, xb_next, ckvb)


def _rope_tables():
    import ml_dtypes
    inv = 1.0 / (ROPE_BASE ** (np.arange(0, R, 2, dtype=np.float32) / R))
    freqs = np.arange(T, dtype=np.float32)[:, None] * inv[None, :]       # [T, 32]
    emb = np.concatenate([freqs, freqs], axis=-1)                         # [T, 64]
    cosT = np.ascontiguousarray(np.cos(emb).T.astype(np.float32))         # [64, T]
    sinT = np.ascontiguousarray(np.sin(emb).T.astype(np.float32))
    cosd = np.concatenate([cosT, cosT], axis=0)                           # [128, T]
    sin_sgn = np.concatenate([-sinT[0:32], sinT[32:64]], axis=0)          # [64, T]
    sind = np.concatenate([sin_sgn, sin_sgn], axis=0)
    return (cosd.astype(ml_dtypes.bfloat16), sind.astype(ml_dtypes.bfloat16))


def host_inmaps(inputs):
    import ml_dtypes
    BF = ml_dtypes.bfloat16

    x = np.asarray(inputs["x"], dtype=np.float32)
    W_dq = np.asarray(inputs["W_dq"], dtype=np.float32)
    W_uq = np.asarray(inputs["W_uq"], dtype=np.float32)
    W_qr = np.asarray(inputs["W_qr"], dtype=np.float32)
    W_dkv = np.asarray(inputs["W_dkv"], dtype=np.float32).astype(BF)
    W_uk = np.asarray(inputs["W_uk"], dtype=np.float32).astype(BF)
    W_uv = np.asarray(inputs["W_uv"], dtype=np.float32).astype(BF)
    W_kr = np.asarray(inputs["W_kr"], dtype=np.float32).astype(BF)
    W_o = np.asarray(inputs["W_o"], dtype=np.float32).astype(BF)

    Aq = (W_dq @ W_uq).astype(BF)     # [C, NH*DH] folded q-content projection
    Aqr = (W_dq @ W_qr).astype(BF)    # [C, NH*R] folded q-rope projection

    cosd, sind = _rope_tables()
    maskv = (np.arange(128)[:, None] <= np.arange(128)[None, :]).astype(
        np.float32).astype(BF)
    onesv = np.ones((128, 128), dtype=np.float32).astype(BF)

    in_maps = []
    for core in range(8):
        b, hg = core // 4, core % 4
        in_maps.append({
            "xT": np.ascontiguousarray(x[b].T).astype(BF),
            "aq": np.ascontiguousarray(Aq[:, 512 * hg:512 * (hg + 1)]),
            "aqr": np.ascontiguousarray(Aqr[:, 256 * hg:256 * (hg + 1)]),
            "wdkv": np.ascontiguousarray(W_dkv[:, 128 * hg:128 * (hg + 1)]),
            "wuk": np.ascontiguousarray(W_uk[:, 512 * hg:512 * (hg + 1)]),
            "wuv": np.ascontiguousarray(W_uv[:, 512 * hg:512 * (hg + 1)]),
            "wkr": np.ascontiguousarray(W_kr[:, 256 * hg:256 * (hg + 1)]),
            "wo": np.ascontiguousarray(W_o[512 * hg:512 * (hg + 1), :]),
            "cosd": cosd,
            "sind": sind,
            "maskd": maskv,
            "onesd": onesv,
        })
    return in_maps


def kernel(**inputs):
    from concourse.bass_utils import run_bass_kernel_spmd

    if "nc" not in _CACHE:
        _CACHE["nc"] = _build_nc()
    nc = _CACHE["nc"]

    in_maps = host_inmaps(inputs)

    res = run_bass_kernel_spmd(nc, in_maps, core_ids=list(range(8)))
    outs = [np.asarray(r["out"]).astype(np.float32) for r in res.results]
    out0 = outs[0] + outs[1] + outs[2] + outs[3]
    out1 = outs[4] + outs[5] + outs[6] + outs[7]
    return np.stack([out0, out1]).astype(np.float32)
